# revision 28
# baseline (speedup 1.0000x reference)
"""3-layer GCN on 8 Trainium2 NeuronCores (Bass/Tile).

Distribution: nodes sharded contiguously across 8 cores (12500 each); edges
partitioned by dst core.  Per layer l:
  table g_l = norm_out * (h_l @ W_l.T)   (row-major fp16, built per-shard,
                                          AllGathered to every core's HBM)
  agg[d]   = sum_{e: dst=d} g_l[src_e]   (dma_gather by src + one-hot
                                          S-matmul segment-sum into PSUM)
  h_{l+1}  = relu((agg + b_l) * norm_in + h_l)   (last layer: no resid/relu)

dma_gather indices are int16, so the gather table is addressed through 4
windows of <=32767 rows (window w = core pair 2w/2w+1's table regions).
Edges are grouped into one gather call per (dst-superblock-of-8-blocks,
window); within a call they are sorted by dst block and padded only at the
call tail (pad slots gather a zero row and carry dst-slot 255, which the
one-hot S kills).  A 128-edge tile may span several dst blocks; for each
(tile, block) pair in the cross-core union a masked one-hot S [128 edges x
128 slots] is built on DVE (batched is_equal against an iota ramp) and
matmul'd into that block's PSUM accumulator.  GpSimd descriptor generation
(~7.7 ns/gathered row) is the critical resource, so the schedule minimizes
gathered rows above all else.

Self-contained: only numpy + concourse (the on-box bass stack).
"""

import numpy as np

N = 100000
D = 128
E = 1600000
NCORES = 8
SHARD = 12500          # nodes per core
NB = 98                # dst blocks of 128 per core (12544 slots, 44 dummies)
ROWSPT = 99            # table rows per partition per core: 98 g-tiles + zero
REGION = 128 * ROWSPT  # 12672 rows per core region
NWIN = 4
WINROWS = 2 * REGION   # 25344 rows per window (2 core regions)
TABLE_ROWS = NCORES * REGION
NSB = 13               # dst superblocks of 8 blocks (last has 2)
SB_BLOCKS = [list(range(sb * 8, min((sb + 1) * 8, NB))) for sb in range(NSB)]
SBATCH = 16            # S one-hot tiles built per DVE op


def _table_row(node):
    node = np.asarray(node)
    c = node // SHARD
    i = node - c * SHARD
    return c * REGION + (i % 128) * ROWSPT + (i // 128)


def preprocess(src, dst):
    """Static schedule + per-core index data from the edge list."""
    src = np.asarray(src).astype(np.int64)
    dst = np.asarray(dst).astype(np.int64)

    deg_out = np.bincount(src, minlength=N).astype(np.float64)
    deg_in = np.bincount(dst, minlength=N).astype(np.float64)
    norm_out = np.clip(deg_out, 1.0, None) ** -0.5
    norm_in = np.clip(deg_in, 1.0, None) ** -0.5

    src_row = _table_row(src)
    win = (src // SHARD) // 2            # window = src core pair
    dst_core = dst // SHARD
    dst_local = dst - dst_core * SHARD
    dst_block = dst_local // 128
    dst_slot = dst_local % 128
    sb_of_block = np.arange(NB) // 8

    # sort edges by (core, superblock, window, block)
    key = (((dst_core * NSB + sb_of_block[dst_block]) * NWIN + win) * NB
           + dst_block)
    order = np.argsort(key, kind="stable")
    s_src_row = src_row[order]
    s_key = key[order]
    s_slot = dst_slot[order]

    # per-(core, sb, w, B) counts
    counts = np.zeros((NCORES, NSB, NWIN, NB), np.int64)
    uk, uc = np.unique(s_key, return_counts=True)
    kc = uk // (NSB * NWIN * NB)
    rem = uk % (NSB * NWIN * NB)
    ksb = rem // (NWIN * NB)
    rem = rem % (NWIN * NB)
    kw = rem // NB
    kb = rem % NB
    counts[kc, ksb, kw, kb] = uc

    call_edges = counts.sum(axis=3)                      # [NCORES, NSB, NWIN]
    call_tiles = (-(-call_edges // 128)).max(axis=0)     # [NSB, NWIN]
    # every block needs >=1 sub so its psum is initialized; guarantee the
    # (sb, 0) call has >=1 tile
    for sb in range(NSB):
        if call_tiles[sb].sum() == 0:
            call_tiles[sb, 0] = 1

    # per-core cumulative start of each (sb, w, B) run inside its call
    run_start = np.cumsum(counts, axis=3) - counts       # [C, NSB, NWIN, NB]

    # union sub schedule: per superblock, BLOCK-MAJOR (a block's matmuls are
    # consecutive so its psum accumulation group never interleaves with its
    # bank-mates').  subs: (ci, t, B) where ci = sb*NWIN + w refs the gather
    # call; sb_spans gives each superblock's sub range.
    for sb in range(NSB):
        if call_tiles[sb].sum() == 0:
            call_tiles[sb, 0] = 1
    subs = []
    sb_spans = []
    for sb in range(NSB):
        lo = len(subs)
        for B in SB_BLOCKS[sb]:
            got = False
            for w in range(NWIN):
                ntile = int(call_tiles[sb, w])
                if ntile == 0:
                    continue
                touched = set()
                for c in range(NCORES):
                    n = counts[c, sb, w, B]
                    if n == 0:
                        continue
                    t0 = int(run_start[c, sb, w, B]) // 128
                    t1 = int(run_start[c, sb, w, B] + n - 1) // 128
                    touched.update(range(t0, t1 + 1))
                for t in sorted(touched):
                    subs.append((sb * NWIN + w, t, B))
                    got = True
            if not got:
                # no edges anywhere for B: one all-zero sub to init psum,
                # referencing the superblock's first non-empty call
                w0 = next(w for w in range(NWIN) if call_tiles[sb, w] > 0)
                subs.append((sb * NWIN + w0, 0, B))
        sb_spans.append((lo, len(subs)))
    NSUB = len(subs)

    # start/stop flags per sub (first/last sub of each block; consecutive)
    first_sub = {}
    last_sub = {}
    for j, (ci, t, B) in enumerate(subs):
        if B not in first_sub:
            first_sub[B] = j
        last_sub[B] = j
    flags = [(B, j == first_sub[B], j == last_sub[B])
             for j, (ci, t, B) in enumerate(subs)]

    # ---- per-core gather indices and per-sub dloc ----
    # map (call, B) -> {tile -> sub j} for dloc scatter
    sub_lut = {}
    for j, (ci, t, B) in enumerate(subs):
        sub_lut[(ci, t, B)] = j

    ci_of = np.empty((NSB, NWIN), np.int64)
    for sb in range(NSB):
        for w in range(NWIN):
            ci_of[sb, w] = sb * NWIN + w
    call_ntile = [int(call_tiles[ci // NWIN, ci % NWIN])
                  for ci in range(NSB * NWIN)]
    tile_base = np.concatenate(
        [[0], np.cumsum(call_ntile)]).astype(np.int64)
    T_total = int(tile_base[-1])

    core_inputs = []
    for c in range(NCORES):
        k_lo = c * NSB * NWIN * NB
        k_hi = (c + 1) * NSB * NWIN * NB
        lo, hi = np.searchsorted(s_key, [k_lo, k_hi])
        ck = s_key[lo:hi] - k_lo
        csb = ck // (NWIN * NB)
        crem = ck % (NWIN * NB)
        cw = crem // NB
        cb = crem % NB
        crow = s_src_row[lo:hi]
        cslot = s_slot[lo:hi]
        # position within the call = run_start[c, sb, w, B] + rank in run
        pos_in_run = np.zeros(hi - lo, np.int64)
        if hi > lo:
            brk = np.flatnonzero(np.diff(ck) != 0) + 1
            starts = np.concatenate([[0], brk])
            lens = np.diff(np.concatenate([starts, [hi - lo]]))
            pos_in_run = np.arange(hi - lo) - np.repeat(starts, lens)
        pos_in_call = run_start[c, csb, cw, cb] + pos_in_run
        tile_in_call = pos_in_call // 128
        p_of_edge = pos_in_call % 128
        cci = ci_of[csb, cw]
        gtile = tile_base[cci] + tile_in_call

        idx16 = np.zeros((T_total, 128), np.int16)
        dloc = np.full((NSUB, 128), 255.0, np.float32)
        # defaults: every slot gathers its window's zero row
        for sb in range(NSB):
            for w in range(NWIN):
                zl = (2 * w) * REGION + (ROWSPT - 1) - w * WINROWS
                ci = int(ci_of[sb, w])
                idx16[tile_base[ci]:tile_base[ci + 1], :] = zl
        idx16[gtile, p_of_edge] = (crow - cw * WINROWS).astype(np.int16)
        sub_j = np.array([sub_lut[(int(a), int(b), int(d))]
                          for a, b, d in zip(cci, tile_in_call, cb)],
                         np.int64)
        dloc[sub_j, p_of_edge] = cslot.astype(np.float32)

        idx_d = np.zeros((128, T_total * 8), np.int16)
        for ci, ntile in enumerate(call_ntile):
            if ntile == 0:
                continue
            t0 = int(tile_base[ci])
            flat = idx16[t0:t0 + ntile].reshape(ntile * 128)
            wrapped = flat.reshape(ntile * 8, 16).T
            idx_d[:, t0 * 8:(t0 + ntile) * 8] = np.tile(wrapped, (8, 1))
        # one-hot S tiles, precomputed: S[p, j*128+s] = (dloc[j, p] == s)
        sdat = np.zeros((NSUB, 128, 128), np.float16)
        jj, pp = np.nonzero(dloc <= 127)
        sdat[jj, pp, dloc[jj, pp].astype(np.int64)] = 1.0
        sdat = np.ascontiguousarray(
            sdat.transpose(1, 0, 2).reshape(128, NSUB * 128))
        core_inputs.append((idx_d, sdat))

    meta = dict(
        T_total=T_total, NSUB=NSUB, subs=subs, flags=flags,
        call_ntile=call_ntile, sb_spans=sb_spans, tile_base=tile_base,
        norm_out=norm_out.astype(np.float32),
        norm_in=norm_in.astype(np.float32),
    )
    return meta, core_inputs


def _slot_vec(vec):
    """[N] per-node vector -> per-core [128, NB] (pad nodes -> 0)."""
    out = []
    for c in range(NCORES):
        a = np.zeros(NB * 128, np.float32)
        a[:SHARD] = vec[c * SHARD:(c + 1) * SHARD]
        out.append(np.ascontiguousarray(a.reshape(NB, 128).T))
    return out


def _slot_rows(mat, dtype):
    """[N, D] rows -> per-core [128, NB*128] (h[p, B*D+f] = row of node
    c*SHARD + B*128 + p)."""
    out = []
    for c in range(NCORES):
        a = np.zeros((NB * 128, D), dtype)
        a[:SHARD] = mat[c * SHARD:(c + 1) * SHARD].astype(dtype)
        out.append(np.ascontiguousarray(
            a.reshape(NB, 128, D).transpose(1, 0, 2).reshape(128, NB * D)))
    return out


def build_program(meta):
    import concourse.mybir as mybir
    import concourse.tile as tile
    import concourse.bacc as bacc
    from concourse.masks import make_identity

    f16 = mybir.dt.float16
    f32 = mybir.dt.float32
    i16 = mybir.dt.int16

    T_total = meta["T_total"]
    NSUB = meta["NSUB"]
    subs = meta["subs"]
    flags = meta["flags"]
    call_ntile = meta["call_ntile"]
    sb_spans = meta["sb_spans"]
    tile_base = meta["tile_base"]

    nc = bacc.Bacc("TRN2", target_bir_lowering=False, debug=False,
                   num_devices=NCORES, num_swdge_queues=4)

    h0_d = nc.dram_tensor("h0", [128, NB * D], f16, kind="ExternalInput")
    idx_d = nc.dram_tensor("gidx", [128, T_total * 8], i16,
                           kind="ExternalInput")
    sdat_d = nc.dram_tensor("sdat", [128, NSUB * 128], f16,
                            kind="ExternalInput")
    no_d = nc.dram_tensor("normout", [128, NB], f32, kind="ExternalInput")
    ni_d = nc.dram_tensor("normin", [128, NB], f32, kind="ExternalInput")
    wt_d = nc.dram_tensor("wt", [D, 3 * D], f16, kind="ExternalInput")
    bb_d = nc.dram_tensor("bb", [128, 12 * D], f16, kind="ExternalInput")
    out_d = nc.dram_tensor("out", [128, NB * D], f16, kind="ExternalOutput")

    g_local = nc.dram_tensor("g_local", [128, ROWSPT * D], f16,
                             kind="Internal")
    table = nc.dram_tensor("gtable", [TABLE_ROWS, D], f16, kind="Internal",
                           addr_space="Shared")

    with tile.TileContext(nc) as tc:
        with (
            tc.tile_pool(name="const", bufs=1) as constp,
            tc.tile_pool(name="ht", bufs=3) as htp,
            tc.tile_pool(name="ix", bufs=8) as ixp,
            tc.tile_pool(name="msgs", bufs=8) as msgp,
            tc.tile_pool(name="sbu", bufs=4) as sp,
            tc.tile_pool(name="cc", bufs=4) as cp,
            tc.tile_pool(name="agg", bufs=4, space="PSUM") as aggp,
            tc.tile_pool(name="pha", bufs=4, space="PSUM") as phap,
        ):
            ident = constp.tile([128, 128], f16)
            make_identity(nc, ident[:])
            h_sb = constp.tile([128, NB * D], f16)
            nc.sync.dma_start(h_sb[:], h0_d.ap())
            no_sb = constp.tile([128, NB], f32)
            nc.sync.dma_start(no_sb[:], no_d.ap())
            ni_sb = constp.tile([128, NB], f32)
            nc.sync.dma_start(ni_sb[:], ni_d.ap())
            wt_sb = constp.tile([128, 3 * D], f16)
            nc.sync.dma_start(wt_sb[:], wt_d.ap())
            bb_sb = constp.tile([128, 12 * D], f16)
            nc.sync.dma_start(bb_sb[:], bb_d.ap())
            stage = constp.tile([128, ROWSPT * D], f16)
            nc.vector.memset(stage[:, NB * D:], 0.0)  # zero rows (t=98)

            mul = mybir.AluOpType.mult
            eq = mybir.AluOpType.is_equal

            def phase_a_quads(l, b_lo, b_hi):
                """Table build g_l = norm_out*(h_l @ W_l.T) for blocks
                [b_lo, b_hi), in quads of 4 blocks per psum bank."""
                for B0 in range(b_lo, b_hi, 4):
                    nb4 = min(4, b_hi - B0)
                    w4 = nb4 * D
                    psT = phap.tile([128, 4 * D], f16, tag="pha",
                                    name=f"psT{l}_{B0}")
                    for j in range(nb4):
                        B = B0 + j
                        nc.tensor.transpose(psT[:, j * D:(j + 1) * D],
                                            h_sb[:, B * D:(B + 1) * D],
                                            ident[:])
                    hT = htp.tile([128, 4 * D], f16, tag="hT",
                                  name=f"hT{l}_{B0}")
                    nc.vector.tensor_copy(hT[:, :w4], psT[:, :w4])
                    psG = phap.tile([128, 4 * D], f32, tag="pha",
                                    name=f"psG{l}_{B0}")
                    for j in range(nb4):
                        nc.tensor.matmul(psG[:, j * D:(j + 1) * D],
                                         lhsT=hT[:, j * D:(j + 1) * D],
                                         rhs=wt_sb[:, l * D:(l + 1) * D],
                                         start=True, stop=True)
                    nc.vector.tensor_tensor(
                        out=stage[:, B0 * D:B0 * D + w4].rearrange(
                            "p (b d) -> p b d", d=D),
                        in0=psG[:, :w4].rearrange("p (b d) -> p b d", d=D),
                        in1=no_sb[:, B0:B0 + nb4].to_broadcast(
                            [128, nb4, D]),
                        op=mul)

            phase_a_quads(0, 0, NB)
            for l in range(3):
                # phase A for this layer was emitted eagerly (per superblock
                # of the previous layer); ship the table.
                nc.sync.dma_start(g_local.ap(), stage[:, :])
                nc.gpsimd.collective_compute(
                    "AllGather", mybir.AluOpType.bypass,
                    replica_groups=[list(range(NCORES))],
                    ins=[g_local.ap()], outs=[table.ap()],
                )
                # ---- phase B: gather + masked-S matmul segment sum ----
                # Per superblock: issue all 4 window gathers (one SWDGE queue
                # each), then run subs BLOCK-MAJOR so each block's psum
                # accumulation group is consecutive (no interleaving within a
                # shared psum bank), then phase C for the superblock.
                sbatch_s0 = -1
                sbatch_tile = None
                for sb in range(NSB):
                    msgs_of = {}
                    for w in range(NWIN):
                        ci = sb * NWIN + w
                        ntile = call_ntile[ci]
                        if ntile == 0:
                            continue
                        ni_call = ntile * 128
                        t0 = int(tile_base[ci])
                        ixt = ixp.tile([128, ntile * 8], i16, tag="ix",
                                       name=f"ix{l}_{ci}")
                        nc.sync.dma_start(
                            ixt[:], idx_d.ap()[:, t0 * 8:(t0 + ntile) * 8])
                        msgs = msgp.tile([128, ntile * D], f16, tag="m",
                                         name=f"m{l}_{ci}")
                        nc.gpsimd.dma_gather(
                            out_ap=msgs[:].rearrange("p (t d) -> p t d", d=D),
                            in_ap=table.ap()[w * WINROWS:(w + 1) * WINROWS, :],
                            idxs_ap=ixt[:],
                            num_idxs=ni_call,
                            num_idxs_reg=ni_call,
                            elem_size=D,
                            single_packet=False,
                            queue_num=w,
                        )
                        msgs_of[w] = msgs
                    nhalf = -(-len(SB_BLOCKS[sb]) // 4)
                    halves = [aggp.tile([128, 4 * D], f32, tag="agg",
                                        name=f"agg{l}_{sb}_{h}")
                              for h in range(nhalf)]
                    slo, shi = sb_spans[sb]
                    for j in range(slo, shi):
                        ci, t, B = subs[j]
                        w = ci % NWIN
                        _, is_first, is_last = flags[j]
                        k = j % SBATCH
                        s0 = (j // SBATCH) * SBATCH
                        if s0 != sbatch_s0:
                            nb2 = min(SBATCH, NSUB - s0)
                            sbatch_tile = sp.tile(
                                [128, SBATCH * 128], f16,
                                tag="S", name=f"S{l}_{s0}")
                            nc.sync.dma_start(
                                sbatch_tile[:, :nb2 * 128],
                                sdat_d.ap()[:, s0 * 128:(s0 + nb2) * 128])
                            sbatch_s0 = s0
                        St = sbatch_tile[:, k * 128:(k + 1) * 128]
                        half, col = (B % 8) // 4, B % 4
                        nc.tensor.matmul(
                            halves[half][:, col * D:(col + 1) * D],
                            lhsT=St,
                            rhs=msgs_of[w][:, t * D:(t + 1) * D],
                            start=is_first, stop=is_last)
                    # ---- phase C for superblock sb ----
                    for h, pa in enumerate(halves):
                        B0 = sb * 8 + h * 4
                        nbh = min(4, NB - B0)
                        wd = nbh * D
                        x1 = cp.tile([128, 4 * D], f16, tag="x1",
                                     name=f"x1{l}_{sb}_{h}")
                        nc.vector.tensor_add(
                            x1[:, :wd], pa[:, :wd],
                            bb_sb[:, l * 4 * D:l * 4 * D + wd])
                        ni_b = ni_sb[:, B0:B0 + nbh].to_broadcast(
                            [128, nbh, D])
                        if l < 2:
                            x2 = cp.tile([128, 4 * D], f16, tag="x2",
                                         name=f"x2{l}_{sb}_{h}")
                            nc.vector.tensor_tensor(
                                out=x2[:, :wd].rearrange(
                                    "p (b d) -> p b d", d=D),
                                in0=x1[:, :wd].rearrange(
                                    "p (b d) -> p b d", d=D),
                                in1=ni_b, op=mul)
                            x3 = cp.tile([128, 4 * D], f16, tag="x3",
                                         name=f"x3{l}_{sb}_{h}")
                            nc.vector.tensor_add(
                                x3[:, :wd], x2[:, :wd],
                                h_sb[:, B0 * D:B0 * D + wd])
                            nc.scalar.activation(
                                h_sb[:, B0 * D:B0 * D + wd], x3[:, :wd],
                                mybir.ActivationFunctionType.Relu)
                        else:
                            nc.vector.tensor_tensor(
                                out=stage[:, B0 * D:B0 * D + wd].rearrange(
                                    "p (b d) -> p b d", d=D),
                                in0=x1[:, :wd].rearrange(
                                    "p (b d) -> p b d", d=D),
                                in1=ni_b, op=mul)
                    if l < 2:
                        # eagerly build next layer's table rows for this sb
                        phase_a_quads(l + 1, sb * 8,
                                      sb * 8 + len(SB_BLOCKS[sb]))
            nc.sync.dma_start(out_d.ap(), stage[:, :NB * D])

    nc.compile()
    return nc


_CACHE = {}


def kernel(feat, src, dst, W1, b1, W2, b2, W3, b3):
    import hashlib
    import concourse.bass_utils as bass_utils

    feat = np.asarray(feat, np.float32)
    src = np.asarray(src)
    dst = np.asarray(dst)
    key = hashlib.sha1(src.tobytes() + dst.tobytes()).hexdigest()
    if key not in _CACHE:
        meta, core_inputs = preprocess(src, dst)
        nc = build_program(meta)
        _CACHE[key] = (meta, core_inputs, nc)
    meta, core_inputs, nc = _CACHE[key]

    Wt = np.concatenate([np.asarray(w, np.float32).T for w in (W1, W2, W3)],
                        axis=1).astype(np.float16)          # [D, 3D]
    bb = np.tile(np.concatenate(
        [np.tile(np.asarray(b, np.float32), 4) for b in (b1, b2, b3)])[None],
        (128, 1)).astype(np.float16)                         # [128, 12D]

    h0_cores = _slot_rows(feat, np.float16)
    no_cores = _slot_vec(meta["norm_out"])
    ni_cores = _slot_vec(meta["norm_in"])

    in_maps = []
    for c in range(NCORES):
        idx_d, sdat = core_inputs[c]
        in_maps.append({
            "h0": h0_cores[c],
            "gidx": idx_d,
            "sdat": sdat,
            "normout": no_cores[c],
            "normin": ni_cores[c],
            "wt": Wt,
            "bb": bb,
        })

    res = bass_utils.run_bass_kernel_spmd(nc, in_maps,
                                          core_ids=list(range(NCORES)))
    out = np.zeros((N, D), np.float32)
    for c in range(NCORES):
        o = res.results[c]["out"].astype(np.float32)
        rows = o.reshape(128, NB, D).transpose(1, 0, 2).reshape(NB * 128, D)
        out[c * SHARD:(c + 1) * SHARD] = rows[:SHARD]
    return out



# revision 34
# speedup vs baseline: 1.2991x; 1.2991x over previous
"""3-layer GCN on 8 Trainium2 NeuronCores (Bass/Tile).

Distribution: nodes sharded contiguously across 8 cores (12500 each); edges
partitioned by dst core.  Per layer l:
  table g_l = norm_out * (h_l @ W_l.T)   (row-major fp16, built per-shard,
                                          AllGathered to every core's HBM)
  agg[d]   = sum_{e: dst=d} g_l[src_e]   (dma_gather by src + one-hot
                                          S-matmul segment-sum into PSUM)
  h_{l+1}  = relu((agg + b_l) * norm_in + h_l)   (last layer: no resid/relu)

dma_gather indices are int16, so the gather table is addressed through 4
windows of <=32767 rows (window w = core pair 2w/2w+1's table regions).
Edges are grouped into one gather call per (dst-superblock-of-8-blocks,
window); within a call they are sorted by dst block and padded only at the
call tail (pad slots gather a zero row and carry dst-slot 255, which the
one-hot S kills).  A 128-edge tile may span several dst blocks; for each
(tile, block) pair in the cross-core union a masked one-hot S [128 edges x
128 slots] is built on DVE (batched is_equal against an iota ramp) and
matmul'd into that block's PSUM accumulator.  GpSimd descriptor generation
(~7.7 ns/gathered row) is the critical resource, so the schedule minimizes
gathered rows above all else.

Self-contained: only numpy + concourse (the on-box bass stack).
"""

import numpy as np

N = 100000
D = 128
E = 1600000
NCORES = 8
SHARD = 12500          # nodes per core
NB = 98                # dst blocks of 128 per core (12544 slots, 44 dummies)
ROWSPT = 99            # table rows per partition per core: 98 g-tiles + zero
REGION = 128 * ROWSPT  # 12672 rows per core region
NWIN = 4
WINROWS = 2 * REGION   # 25344 rows per window (2 core regions)
TABLE_ROWS = NCORES * REGION
NSB = 13               # dst superblocks of 8 blocks (last has 2)
SB_BLOCKS = [list(range(sb * 8, min((sb + 1) * 8, NB))) for sb in range(NSB)]
SBATCH = 16            # S one-hot tiles built per DVE op


def _table_row(node):
    node = np.asarray(node)
    c = node // SHARD
    i = node - c * SHARD
    return c * REGION + (i % 128) * ROWSPT + (i // 128)


def preprocess(src, dst):
    """Static schedule + per-core index data from the edge list."""
    src = np.asarray(src).astype(np.int64)
    dst = np.asarray(dst).astype(np.int64)

    deg_out = np.bincount(src, minlength=N).astype(np.float64)
    deg_in = np.bincount(dst, minlength=N).astype(np.float64)
    norm_out = np.clip(deg_out, 1.0, None) ** -0.5
    norm_in = np.clip(deg_in, 1.0, None) ** -0.5

    src_row = _table_row(src)
    win = (src // SHARD) // 2            # window = src core pair
    dst_core = dst // SHARD
    dst_local = dst - dst_core * SHARD
    dst_block = dst_local // 128
    dst_slot = dst_local % 128
    sb_of_block = np.arange(NB) // 8

    # sort edges by (core, superblock, window, block)
    key = (((dst_core * NSB + sb_of_block[dst_block]) * NWIN + win) * NB
           + dst_block)
    order = np.argsort(key, kind="stable")
    s_src_row = src_row[order]
    s_key = key[order]
    s_slot = dst_slot[order]

    # per-(core, sb, w, B) counts
    counts = np.zeros((NCORES, NSB, NWIN, NB), np.int64)
    uk, uc = np.unique(s_key, return_counts=True)
    kc = uk // (NSB * NWIN * NB)
    rem = uk % (NSB * NWIN * NB)
    ksb = rem // (NWIN * NB)
    rem = rem % (NWIN * NB)
    kw = rem // NB
    kb = rem % NB
    counts[kc, ksb, kw, kb] = uc

    call_edges = counts.sum(axis=3)                      # [NCORES, NSB, NWIN]
    call_tiles = (-(-call_edges // 128)).max(axis=0)     # [NSB, NWIN]
    # every block needs >=1 sub so its psum is initialized; guarantee the
    # (sb, 0) call has >=1 tile
    for sb in range(NSB):
        if call_tiles[sb].sum() == 0:
            call_tiles[sb, 0] = 1

    # per-core cumulative start of each (sb, w, B) run inside its call
    run_start = np.cumsum(counts, axis=3) - counts       # [C, NSB, NWIN, NB]

    # union sub schedule: per superblock, BLOCK-MAJOR (a block's matmuls are
    # consecutive so its psum accumulation group never interleaves with its
    # bank-mates').  subs: (ci, t, B) where ci = sb*NWIN + w refs the gather
    # call; sb_spans gives each superblock's sub range.
    for sb in range(NSB):
        if call_tiles[sb].sum() == 0:
            call_tiles[sb, 0] = 1
    subs = []
    sb_spans = []
    for sb in range(NSB):
        lo = len(subs)
        for B in SB_BLOCKS[sb]:
            got = False
            for w in range(NWIN):
                ntile = int(call_tiles[sb, w])
                if ntile == 0:
                    continue
                touched = set()
                for c in range(NCORES):
                    n = counts[c, sb, w, B]
                    if n == 0:
                        continue
                    t0 = int(run_start[c, sb, w, B]) // 128
                    t1 = int(run_start[c, sb, w, B] + n - 1) // 128
                    touched.update(range(t0, t1 + 1))
                for t in sorted(touched):
                    subs.append((sb * NWIN + w, t, B))
                    got = True
            if not got:
                # no edges anywhere for B: one all-zero sub to init psum,
                # referencing the superblock's first non-empty call
                w0 = next(w for w in range(NWIN) if call_tiles[sb, w] > 0)
                subs.append((sb * NWIN + w0, 0, B))
        sb_spans.append((lo, len(subs)))
    NSUB = len(subs)

    # start/stop flags per sub (first/last sub of each block; consecutive)
    first_sub = {}
    last_sub = {}
    for j, (ci, t, B) in enumerate(subs):
        if B not in first_sub:
            first_sub[B] = j
        last_sub[B] = j
    flags = [(B, j == first_sub[B], j == last_sub[B])
             for j, (ci, t, B) in enumerate(subs)]

    # ---- per-core gather indices and per-sub dloc ----
    # map (call, B) -> {tile -> sub j} for dloc scatter
    sub_lut = {}
    for j, (ci, t, B) in enumerate(subs):
        sub_lut[(ci, t, B)] = j

    ci_of = np.empty((NSB, NWIN), np.int64)
    for sb in range(NSB):
        for w in range(NWIN):
            ci_of[sb, w] = sb * NWIN + w
    call_ntile = [int(call_tiles[ci // NWIN, ci % NWIN])
                  for ci in range(NSB * NWIN)]
    tile_base = np.concatenate(
        [[0], np.cumsum(call_ntile)]).astype(np.int64)
    T_total = int(tile_base[-1])

    core_inputs = []
    for c in range(NCORES):
        k_lo = c * NSB * NWIN * NB
        k_hi = (c + 1) * NSB * NWIN * NB
        lo, hi = np.searchsorted(s_key, [k_lo, k_hi])
        ck = s_key[lo:hi] - k_lo
        csb = ck // (NWIN * NB)
        crem = ck % (NWIN * NB)
        cw = crem // NB
        cb = crem % NB
        crow = s_src_row[lo:hi]
        cslot = s_slot[lo:hi]
        # position within the call = run_start[c, sb, w, B] + rank in run
        pos_in_run = np.zeros(hi - lo, np.int64)
        if hi > lo:
            brk = np.flatnonzero(np.diff(ck) != 0) + 1
            starts = np.concatenate([[0], brk])
            lens = np.diff(np.concatenate([starts, [hi - lo]]))
            pos_in_run = np.arange(hi - lo) - np.repeat(starts, lens)
        pos_in_call = run_start[c, csb, cw, cb] + pos_in_run
        tile_in_call = pos_in_call // 128
        p_of_edge = pos_in_call % 128
        cci = ci_of[csb, cw]
        gtile = tile_base[cci] + tile_in_call

        idx16 = np.zeros((T_total, 128), np.int16)
        dloc = np.full((NSUB, 128), 255.0, np.float32)
        # defaults: every slot gathers its window's zero row
        for sb in range(NSB):
            for w in range(NWIN):
                zl = (2 * w) * REGION + (ROWSPT - 1) - w * WINROWS
                ci = int(ci_of[sb, w])
                idx16[tile_base[ci]:tile_base[ci + 1], :] = zl
        idx16[gtile, p_of_edge] = (crow - cw * WINROWS).astype(np.int16)
        sub_j = np.array([sub_lut[(int(a), int(b), int(d))]
                          for a, b, d in zip(cci, tile_in_call, cb)],
                         np.int64)
        dloc[sub_j, p_of_edge] = cslot.astype(np.float32)

        idx_d = np.zeros((128, T_total * 8), np.int16)
        for ci, ntile in enumerate(call_ntile):
            if ntile == 0:
                continue
            t0 = int(tile_base[ci])
            flat = idx16[t0:t0 + ntile].reshape(ntile * 128)
            wrapped = flat.reshape(ntile * 8, 16).T
            idx_d[:, t0 * 8:(t0 + ntile) * 8] = np.tile(wrapped, (8, 1))
        # per-sub dst-slot of each edge row (int8; -1 = inactive)
        dloc8 = np.where(dloc <= 127, dloc, -1).astype(np.int8)
        dloc_d = np.ascontiguousarray(dloc8.T)          # [128, NSUB]
        core_inputs.append((idx_d, dloc_d))

    meta = dict(
        T_total=T_total, NSUB=NSUB, subs=subs, flags=flags,
        call_ntile=call_ntile, sb_spans=sb_spans, tile_base=tile_base,
        norm_out=norm_out.astype(np.float32),
        norm_in=norm_in.astype(np.float32),
    )
    return meta, core_inputs


def _slot_vec(vec):
    """[N] per-node vector -> per-core [128, NB] (pad nodes -> 0)."""
    out = []
    for c in range(NCORES):
        a = np.zeros(NB * 128, np.float32)
        a[:SHARD] = vec[c * SHARD:(c + 1) * SHARD]
        out.append(np.ascontiguousarray(a.reshape(NB, 128).T))
    return out


def _slot_rows(mat, dtype):
    """[N, D] rows -> per-core [128, NB*128] (h[p, B*D+f] = row of node
    c*SHARD + B*128 + p)."""
    out = []
    for c in range(NCORES):
        a = np.zeros((NB * 128, D), dtype)
        a[:SHARD] = mat[c * SHARD:(c + 1) * SHARD].astype(dtype)
        out.append(np.ascontiguousarray(
            a.reshape(NB, 128, D).transpose(1, 0, 2).reshape(128, NB * D)))
    return out


def build_program(meta):
    import concourse.mybir as mybir
    import concourse.tile as tile
    import concourse.bacc as bacc
    from concourse.masks import make_identity

    f16 = mybir.dt.float16
    f32 = mybir.dt.float32
    i16 = mybir.dt.int16

    T_total = meta["T_total"]
    NSUB = meta["NSUB"]
    subs = meta["subs"]
    flags = meta["flags"]
    call_ntile = meta["call_ntile"]
    sb_spans = meta["sb_spans"]
    tile_base = meta["tile_base"]

    nc = bacc.Bacc("TRN2", target_bir_lowering=False, debug=False,
                   num_devices=NCORES, num_swdge_queues=4)

    h0_d = nc.dram_tensor("h0", [128, NB * D], f16, kind="ExternalInput")
    idx_d = nc.dram_tensor("gidx", [128, T_total * 8], i16,
                           kind="ExternalInput")
    i8 = mybir.dt.int8
    dloc_d = nc.dram_tensor("dloc", [128, NSUB], i8, kind="ExternalInput")
    no_d = nc.dram_tensor("normout", [128, NB], f32, kind="ExternalInput")
    ni_d = nc.dram_tensor("normin", [128, NB], f32, kind="ExternalInput")
    wt_d = nc.dram_tensor("wt", [D, 3 * D], f16, kind="ExternalInput")
    bb_d = nc.dram_tensor("bb", [128, 12 * D], f16, kind="ExternalInput")
    out_d = nc.dram_tensor("out", [128, NB * D], f16, kind="ExternalOutput")

    g_local = nc.dram_tensor("g_local", [128, ROWSPT * D], f16,
                             kind="Internal")
    table = nc.dram_tensor("gtable", [TABLE_ROWS, D], f16, kind="Internal",
                           addr_space="Shared")

    with tile.TileContext(nc) as tc:
        with (
            tc.tile_pool(name="const", bufs=1) as constp,
            tc.tile_pool(name="ht", bufs=3) as htp,
            tc.tile_pool(name="ix", bufs=8) as ixp,
            tc.tile_pool(name="msgs", bufs=8) as msgp,
            tc.tile_pool(name="sbu", bufs=4) as sp,
            tc.tile_pool(name="cc", bufs=4) as cp,
            tc.tile_pool(name="agg", bufs=4, space="PSUM") as aggp,
            tc.tile_pool(name="pha", bufs=4, space="PSUM") as phap,
        ):
            ident = constp.tile([128, 128], f16)
            make_identity(nc, ident[:])
            iota_i = constp.tile([128, SBATCH * 128], i16)
            nc.gpsimd.iota(iota_i[:], pattern=[[0, SBATCH], [1, 128]], base=0,
                           channel_multiplier=0)
            iota_8 = constp.tile([128, SBATCH * 128], i8)
            nc.vector.tensor_copy(iota_8[:], iota_i[:])
            h_sb = constp.tile([128, NB * D], f16)
            nc.sync.dma_start(h_sb[:], h0_d.ap())
            dloc_sb = constp.tile([128, NSUB], i8)
            nc.sync.dma_start(dloc_sb[:], dloc_d.ap())
            no_sb = constp.tile([128, NB], f32)
            nc.sync.dma_start(no_sb[:], no_d.ap())
            ni_sb = constp.tile([128, NB], f32)
            nc.sync.dma_start(ni_sb[:], ni_d.ap())
            wt_sb = constp.tile([128, 3 * D], f16)
            nc.sync.dma_start(wt_sb[:], wt_d.ap())
            bb_sb = constp.tile([128, 12 * D], f16)
            nc.sync.dma_start(bb_sb[:], bb_d.ap())
            stage = constp.tile([128, ROWSPT * D], f16)
            nc.vector.memset(stage[:, NB * D:], 0.0)  # zero rows (t=98)

            mul = mybir.AluOpType.mult
            eq = mybir.AluOpType.is_equal

            def phase_a_quads(l, b_lo, b_hi):
                """Table build g_l = norm_out*(h_l @ W_l.T) for blocks
                [b_lo, b_hi), in quads of 4 blocks per psum bank."""
                for B0 in range(b_lo, b_hi, 4):
                    nb4 = min(4, b_hi - B0)
                    w4 = nb4 * D
                    psT = phap.tile([128, 4 * D], f16, tag="pha",
                                    name=f"psT{l}_{B0}")
                    for j in range(nb4):
                        B = B0 + j
                        nc.tensor.transpose(psT[:, j * D:(j + 1) * D],
                                            h_sb[:, B * D:(B + 1) * D],
                                            ident[:])
                    hT = htp.tile([128, 4 * D], f16, tag="hT",
                                  name=f"hT{l}_{B0}")
                    nc.vector.tensor_copy(hT[:, :w4], psT[:, :w4])
                    psG = phap.tile([128, 4 * D], f32, tag="pha",
                                    name=f"psG{l}_{B0}")
                    for j in range(nb4):
                        nc.tensor.matmul(psG[:, j * D:(j + 1) * D],
                                         lhsT=hT[:, j * D:(j + 1) * D],
                                         rhs=wt_sb[:, l * D:(l + 1) * D],
                                         start=True, stop=True)
                    nc.vector.tensor_tensor(
                        out=stage[:, B0 * D:B0 * D + w4].rearrange(
                            "p (b d) -> p b d", d=D),
                        in0=psG[:, :w4].rearrange("p (b d) -> p b d", d=D),
                        in1=no_sb[:, B0:B0 + nb4].to_broadcast(
                            [128, nb4, D]),
                        op=mul)

            phase_a_quads(0, 0, NB)
            for l in range(3):
                # phase A for this layer was emitted eagerly (per superblock
                # of the previous layer); ship the table.
                nc.sync.dma_start(g_local.ap(), stage[:, :])
                nc.gpsimd.collective_compute(
                    "AllGather", mybir.AluOpType.bypass,
                    replica_groups=[list(range(NCORES))],
                    ins=[g_local.ap()], outs=[table.ap()],
                )
                # ---- phase B: gather + masked-S matmul segment sum ----
                # Per superblock: issue all 4 window gathers (one SWDGE queue
                # each), then run subs BLOCK-MAJOR so each block's psum
                # accumulation group is consecutive (no interleaving within a
                # shared psum bank), then phase C for the superblock.
                sbatch_s0 = -1
                sbatch_tile = None
                for sb in range(NSB):
                    msgs_of = {}
                    for w in range(NWIN):
                        ci = sb * NWIN + w
                        ntile = call_ntile[ci]
                        if ntile == 0:
                            continue
                        ni_call = ntile * 128
                        t0 = int(tile_base[ci])
                        ixt = ixp.tile([128, ntile * 8], i16, tag="ix",
                                       name=f"ix{l}_{ci}")
                        nc.sync.dma_start(
                            ixt[:], idx_d.ap()[:, t0 * 8:(t0 + ntile) * 8])
                        msgs = msgp.tile([128, ntile * D], f16, tag="m",
                                         name=f"m{l}_{ci}")
                        nc.gpsimd.dma_gather(
                            out_ap=msgs[:].rearrange("p (t d) -> p t d", d=D),
                            in_ap=table.ap()[w * WINROWS:(w + 1) * WINROWS, :],
                            idxs_ap=ixt[:],
                            num_idxs=ni_call,
                            num_idxs_reg=ni_call,
                            elem_size=D,
                            single_packet=False,
                            queue_num=w,
                        )
                        msgs_of[w] = msgs
                    nhalf = -(-len(SB_BLOCKS[sb]) // 4)
                    halves = [aggp.tile([128, 4 * D], f32, tag="agg",
                                        name=f"agg{l}_{sb}_{h}")
                              for h in range(nhalf)]
                    slo, shi = sb_spans[sb]
                    for j in range(slo, shi):
                        ci, t, B = subs[j]
                        w = ci % NWIN
                        _, is_first, is_last = flags[j]
                        k = j % SBATCH
                        s0 = (j // SBATCH) * SBATCH
                        if s0 != sbatch_s0:
                            nb2 = min(SBATCH, NSUB - s0)
                            sbatch_tile = sp.tile(
                                [128, SBATCH * 128], f16,
                                tag="S", name=f"S{l}_{s0}")
                            nc.vector.tensor_tensor(
                                out=sbatch_tile[:, :nb2 * 128].rearrange(
                                    "p (a b) -> p a b", b=128),
                                in0=iota_8[:, :nb2 * 128].rearrange(
                                    "p (a b) -> p a b", b=128),
                                in1=dloc_sb[:, s0:s0 + nb2].to_broadcast(
                                    [128, nb2, 128]),
                                op=eq)
                            sbatch_s0 = s0
                        St = sbatch_tile[:, k * 128:(k + 1) * 128]
                        half, col = (B % 8) // 4, B % 4
                        nc.tensor.matmul(
                            halves[half][:, col * D:(col + 1) * D],
                            lhsT=St,
                            rhs=msgs_of[w][:, t * D:(t + 1) * D],
                            start=is_first, stop=is_last)
                    # ---- phase C for superblock sb ----
                    for h, pa in enumerate(halves):
                        B0 = sb * 8 + h * 4
                        nbh = min(4, NB - B0)
                        wd = nbh * D
                        x1 = cp.tile([128, 4 * D], f16, tag="x1",
                                     name=f"x1{l}_{sb}_{h}")
                        nc.vector.tensor_add(
                            x1[:, :wd], pa[:, :wd],
                            bb_sb[:, l * 4 * D:l * 4 * D + wd])
                        ni_b = ni_sb[:, B0:B0 + nbh].to_broadcast(
                            [128, nbh, D])
                        if l < 2:
                            x2 = cp.tile([128, 4 * D], f16, tag="x2",
                                         name=f"x2{l}_{sb}_{h}")
                            nc.vector.tensor_tensor(
                                out=x2[:, :wd].rearrange(
                                    "p (b d) -> p b d", d=D),
                                in0=x1[:, :wd].rearrange(
                                    "p (b d) -> p b d", d=D),
                                in1=ni_b, op=mul)
                            x3 = cp.tile([128, 4 * D], f16, tag="x3",
                                         name=f"x3{l}_{sb}_{h}")
                            nc.vector.tensor_add(
                                x3[:, :wd], x2[:, :wd],
                                h_sb[:, B0 * D:B0 * D + wd])
                            nc.scalar.activation(
                                h_sb[:, B0 * D:B0 * D + wd], x3[:, :wd],
                                mybir.ActivationFunctionType.Relu)
                        else:
                            nc.vector.tensor_tensor(
                                out=stage[:, B0 * D:B0 * D + wd].rearrange(
                                    "p (b d) -> p b d", d=D),
                                in0=x1[:, :wd].rearrange(
                                    "p (b d) -> p b d", d=D),
                                in1=ni_b, op=mul)
                    if l < 2:
                        # eagerly build next layer's table rows for this sb
                        phase_a_quads(l + 1, sb * 8,
                                      sb * 8 + len(SB_BLOCKS[sb]))
            nc.sync.dma_start(out_d.ap(), stage[:, :NB * D])

    nc.compile()
    return nc


_CACHE = {}


def kernel(feat, src, dst, W1, b1, W2, b2, W3, b3):
    import hashlib
    import concourse.bass_utils as bass_utils

    feat = np.asarray(feat, np.float32)
    src = np.asarray(src)
    dst = np.asarray(dst)
    key = hashlib.sha1(src.tobytes() + dst.tobytes()).hexdigest()
    if key not in _CACHE:
        meta, core_inputs = preprocess(src, dst)
        nc = build_program(meta)
        _CACHE[key] = (meta, core_inputs, nc)
    meta, core_inputs, nc = _CACHE[key]

    Wt = np.concatenate([np.asarray(w, np.float32).T for w in (W1, W2, W3)],
                        axis=1).astype(np.float16)          # [D, 3D]
    bb = np.tile(np.concatenate(
        [np.tile(np.asarray(b, np.float32), 4) for b in (b1, b2, b3)])[None],
        (128, 1)).astype(np.float16)                         # [128, 12D]

    h0_cores = _slot_rows(feat, np.float16)
    no_cores = _slot_vec(meta["norm_out"])
    ni_cores = _slot_vec(meta["norm_in"])

    in_maps = []
    for c in range(NCORES):
        idx_d, dloc_d = core_inputs[c]
        in_maps.append({
            "h0": h0_cores[c],
            "gidx": idx_d,
            "dloc": dloc_d,
            "normout": no_cores[c],
            "normin": ni_cores[c],
            "wt": Wt,
            "bb": bb,
        })

    res = bass_utils.run_bass_kernel_spmd(nc, in_maps,
                                          core_ids=list(range(NCORES)))
    out = np.zeros((N, D), np.float32)
    for c in range(NCORES):
        o = res.results[c]["out"].astype(np.float32)
        rows = o.reshape(128, NB, D).transpose(1, 0, 2).reshape(NB * 128, D)
        out[c * SHARD:(c + 1) * SHARD] = rows[:SHARD]
    return out



# revision 41
# speedup vs baseline: 1.3315x; 1.0250x over previous
"""3-layer GCN on 8 Trainium2 NeuronCores (Bass/Tile).

Distribution: nodes sharded contiguously across 8 cores (12500 each); edges
partitioned by dst core.  Per layer l:
  table g_l = norm_out * (h_l @ W_l.T)   (row-major fp16, built per-shard,
                                          AllGathered to every core's HBM)
  agg[d]   = sum_{e: dst=d} g_l[src_e]   (dma_gather by src + one-hot
                                          S-matmul segment-sum into PSUM)
  h_{l+1}  = relu((agg + b_l) * norm_in + h_l)   (last layer: no resid/relu)

dma_gather indices are int16, so the gather table is addressed through 4
windows of <=32767 rows (window w = core pair 2w/2w+1's table regions).
Edges are grouped into one gather call per (dst-superblock-of-8-blocks,
window); within a call they are sorted by dst block and padded only at the
call tail (pad slots gather a zero row and carry dst-slot 255, which the
one-hot S kills).  A 128-edge tile may span several dst blocks; for each
(tile, block) pair in the cross-core union a masked one-hot S [128 edges x
128 slots] is built on DVE (batched is_equal against an iota ramp) and
matmul'd into that block's PSUM accumulator.  GpSimd descriptor generation
(~7.7 ns/gathered row) is the critical resource, so the schedule minimizes
gathered rows above all else.

Self-contained: only numpy + concourse (the on-box bass stack).
"""

import numpy as np

N = 100000
D = 128
E = 1600000
NCORES = 8
SHARD = 12500          # nodes per core
NB = 98                # dst blocks of 128 per core (12544 slots, 44 dummies)
ROWSPT = 99            # table rows per partition per core: 98 g-tiles + zero
REGION = 128 * ROWSPT  # 12672 rows per core region
NWIN = 4
WINROWS = 2 * REGION   # 25344 rows per window (2 core regions)
TABLE_ROWS = NCORES * REGION
NSB = 13               # dst superblocks of 8 blocks (last has 2)
SB_BLOCKS = [list(range(sb * 8, min((sb + 1) * 8, NB))) for sb in range(NSB)]
SBATCH = 32            # S one-hot tiles per HBM-stream DMA


def _table_row(node):
    node = np.asarray(node)
    c = node // SHARD
    i = node - c * SHARD
    return c * REGION + (i % 128) * ROWSPT + (i // 128)


def preprocess(src, dst):
    """Static schedule + per-core index data from the edge list."""
    src = np.asarray(src).astype(np.int64)
    dst = np.asarray(dst).astype(np.int64)

    deg_out = np.bincount(src, minlength=N).astype(np.float64)
    deg_in = np.bincount(dst, minlength=N).astype(np.float64)
    norm_out = np.clip(deg_out, 1.0, None) ** -0.5
    norm_in = np.clip(deg_in, 1.0, None) ** -0.5

    src_row = _table_row(src)
    win = (src // SHARD) // 2            # window = src core pair
    dst_core = dst // SHARD
    dst_local = dst - dst_core * SHARD
    dst_block = dst_local // 128
    dst_slot = dst_local % 128
    sb_of_block = np.arange(NB) // 8

    # sort edges by (core, superblock, window, block)
    key = (((dst_core * NSB + sb_of_block[dst_block]) * NWIN + win) * NB
           + dst_block)
    order = np.argsort(key, kind="stable")
    s_src_row = src_row[order]
    s_key = key[order]
    s_slot = dst_slot[order]

    # per-(core, sb, w, B) counts
    counts = np.zeros((NCORES, NSB, NWIN, NB), np.int64)
    uk, uc = np.unique(s_key, return_counts=True)
    kc = uk // (NSB * NWIN * NB)
    rem = uk % (NSB * NWIN * NB)
    ksb = rem // (NWIN * NB)
    rem = rem % (NWIN * NB)
    kw = rem // NB
    kb = rem % NB
    counts[kc, ksb, kw, kb] = uc

    call_edges = counts.sum(axis=3)                      # [NCORES, NSB, NWIN]
    call_tiles = (-(-call_edges // 128)).max(axis=0)     # [NSB, NWIN]
    # every block needs >=1 sub so its psum is initialized; guarantee the
    # (sb, 0) call has >=1 tile
    for sb in range(NSB):
        if call_tiles[sb].sum() == 0:
            call_tiles[sb, 0] = 1

    # per-core cumulative start of each (sb, w, B) run inside its call
    run_start = np.cumsum(counts, axis=3) - counts       # [C, NSB, NWIN, NB]

    # union sub schedule: per superblock, BLOCK-MAJOR (a block's matmuls are
    # consecutive so its psum accumulation group never interleaves with its
    # bank-mates').  subs: (ci, t, B) where ci = sb*NWIN + w refs the gather
    # call; sb_spans gives each superblock's sub range.
    for sb in range(NSB):
        if call_tiles[sb].sum() == 0:
            call_tiles[sb, 0] = 1
    subs = []
    sb_spans = []
    for sb in range(NSB):
        lo = len(subs)
        for B in SB_BLOCKS[sb]:
            got = False
            for w in range(NWIN):
                ntile = int(call_tiles[sb, w])
                if ntile == 0:
                    continue
                touched = set()
                for c in range(NCORES):
                    n = counts[c, sb, w, B]
                    if n == 0:
                        continue
                    t0 = int(run_start[c, sb, w, B]) // 128
                    t1 = int(run_start[c, sb, w, B] + n - 1) // 128
                    touched.update(range(t0, t1 + 1))
                for t in sorted(touched):
                    subs.append((sb * NWIN + w, t, B))
                    got = True
            if not got:
                # no edges anywhere for B: one all-zero sub to init psum,
                # referencing the superblock's first non-empty call
                w0 = next(w for w in range(NWIN) if call_tiles[sb, w] > 0)
                subs.append((sb * NWIN + w0, 0, B))
        sb_spans.append((lo, len(subs)))
    NSUB = len(subs)

    # start/stop flags per sub (first/last sub of each block; consecutive)
    first_sub = {}
    last_sub = {}
    for j, (ci, t, B) in enumerate(subs):
        if B not in first_sub:
            first_sub[B] = j
        last_sub[B] = j
    flags = [(B, j == first_sub[B], j == last_sub[B])
             for j, (ci, t, B) in enumerate(subs)]

    # ---- per-core gather indices and per-sub dloc ----
    # map (call, B) -> {tile -> sub j} for dloc scatter
    sub_lut = {}
    for j, (ci, t, B) in enumerate(subs):
        sub_lut[(ci, t, B)] = j

    ci_of = np.empty((NSB, NWIN), np.int64)
    for sb in range(NSB):
        for w in range(NWIN):
            ci_of[sb, w] = sb * NWIN + w
    call_ntile = [int(call_tiles[ci // NWIN, ci % NWIN])
                  for ci in range(NSB * NWIN)]
    tile_base = np.concatenate(
        [[0], np.cumsum(call_ntile)]).astype(np.int64)
    T_total = int(tile_base[-1])

    core_inputs = []
    for c in range(NCORES):
        k_lo = c * NSB * NWIN * NB
        k_hi = (c + 1) * NSB * NWIN * NB
        lo, hi = np.searchsorted(s_key, [k_lo, k_hi])
        ck = s_key[lo:hi] - k_lo
        csb = ck // (NWIN * NB)
        crem = ck % (NWIN * NB)
        cw = crem // NB
        cb = crem % NB
        crow = s_src_row[lo:hi]
        cslot = s_slot[lo:hi]
        # position within the call = run_start[c, sb, w, B] + rank in run
        pos_in_run = np.zeros(hi - lo, np.int64)
        if hi > lo:
            brk = np.flatnonzero(np.diff(ck) != 0) + 1
            starts = np.concatenate([[0], brk])
            lens = np.diff(np.concatenate([starts, [hi - lo]]))
            pos_in_run = np.arange(hi - lo) - np.repeat(starts, lens)
        pos_in_call = run_start[c, csb, cw, cb] + pos_in_run
        tile_in_call = pos_in_call // 128
        p_of_edge = pos_in_call % 128
        cci = ci_of[csb, cw]
        gtile = tile_base[cci] + tile_in_call

        idx16 = np.zeros((T_total, 128), np.int16)
        dloc = np.full((NSUB, 128), 255.0, np.float32)
        # defaults: every slot gathers its window's zero row
        for sb in range(NSB):
            for w in range(NWIN):
                zl = (2 * w) * REGION + (ROWSPT - 1) - w * WINROWS
                ci = int(ci_of[sb, w])
                idx16[tile_base[ci]:tile_base[ci + 1], :] = zl
        idx16[gtile, p_of_edge] = (crow - cw * WINROWS).astype(np.int16)
        sub_j = np.array([sub_lut[(int(a), int(b), int(d))]
                          for a, b, d in zip(cci, tile_in_call, cb)],
                         np.int64)
        dloc[sub_j, p_of_edge] = cslot.astype(np.float32)

        idx_d = np.zeros((128, T_total * 8), np.int16)
        for ci, ntile in enumerate(call_ntile):
            if ntile == 0:
                continue
            t0 = int(tile_base[ci])
            flat = idx16[t0:t0 + ntile].reshape(ntile * 128)
            wrapped = flat.reshape(ntile * 8, 16).T
            idx_d[:, t0 * 8:(t0 + ntile) * 8] = np.tile(wrapped, (8, 1))
        # one-hot S tiles precomputed as fp8e4m3 (1.0 = 0x38):
        # S[p, j*128+s] = (dloc[j, p] == s)
        import ml_dtypes
        sdat = np.zeros((NSUB, 128, 128), np.uint8)
        jj, pp = np.nonzero(dloc <= 127)
        sdat[jj, pp, dloc[jj, pp].astype(np.int64)] = 0x38
        sdat = np.ascontiguousarray(
            sdat.transpose(1, 0, 2).reshape(128, NSUB * 128)).view(
            ml_dtypes.float8_e4m3)
        core_inputs.append((idx_d, sdat))

    meta = dict(
        T_total=T_total, NSUB=NSUB, subs=subs, flags=flags,
        call_ntile=call_ntile, sb_spans=sb_spans, tile_base=tile_base,
        norm_out=norm_out.astype(np.float32),
        norm_in=norm_in.astype(np.float32),
    )
    return meta, core_inputs


def _slot_vec(vec):
    """[N] per-node vector -> per-core [128, NB] (pad nodes -> 0)."""
    out = []
    for c in range(NCORES):
        a = np.zeros(NB * 128, np.float32)
        a[:SHARD] = vec[c * SHARD:(c + 1) * SHARD]
        out.append(np.ascontiguousarray(a.reshape(NB, 128).T))
    return out


def _slot_rows(mat, dtype):
    """[N, D] rows -> per-core [128, NB*128] (h[p, B*D+f] = row of node
    c*SHARD + B*128 + p)."""
    out = []
    for c in range(NCORES):
        a = np.zeros((NB * 128, D), dtype)
        a[:SHARD] = mat[c * SHARD:(c + 1) * SHARD].astype(dtype)
        out.append(np.ascontiguousarray(
            a.reshape(NB, 128, D).transpose(1, 0, 2).reshape(128, NB * D)))
    return out


def build_program(meta):
    import concourse.mybir as mybir
    import concourse.tile as tile
    import concourse.bacc as bacc
    from concourse.masks import make_identity

    f16 = mybir.dt.float16
    f32 = mybir.dt.float32
    i16 = mybir.dt.int16

    T_total = meta["T_total"]
    NSUB = meta["NSUB"]
    subs = meta["subs"]
    flags = meta["flags"]
    call_ntile = meta["call_ntile"]
    sb_spans = meta["sb_spans"]
    tile_base = meta["tile_base"]

    nc = bacc.Bacc("TRN2", target_bir_lowering=False, debug=False,
                   num_devices=NCORES, num_swdge_queues=4)

    h0_d = nc.dram_tensor("h0", [128, NB * D], f16, kind="ExternalInput")
    idx_d = nc.dram_tensor("gidx", [128, T_total * 8], i16,
                           kind="ExternalInput")
    f8 = mybir.dt.float8e4
    sdat_d = nc.dram_tensor("sdat", [128, NSUB * 128], f8,
                            kind="ExternalInput")
    no_d = nc.dram_tensor("normout", [128, NB], f32, kind="ExternalInput")
    ni_d = nc.dram_tensor("normin", [128, NB], f32, kind="ExternalInput")
    wt_d = nc.dram_tensor("wt", [D, 3 * D], f16, kind="ExternalInput")
    bb_d = nc.dram_tensor("bb", [128, 12 * D], f16, kind="ExternalInput")
    out_d = nc.dram_tensor("out", [128, NB * D], f16, kind="ExternalOutput")

    g_local = nc.dram_tensor("g_local", [128, ROWSPT * D], f16,
                             kind="Internal")
    table = nc.dram_tensor("gtable", [TABLE_ROWS, D], f16, kind="Internal",
                           addr_space="Shared")

    with tile.TileContext(nc) as tc:
        with (
            tc.tile_pool(name="const", bufs=1) as constp,
            tc.tile_pool(name="ht", bufs=3) as htp,
            tc.tile_pool(name="ix", bufs=8) as ixp,
            tc.tile_pool(name="msgs", bufs=8) as msgp,
            tc.tile_pool(name="sbu", bufs=4) as sp,
            tc.tile_pool(name="cc", bufs=4) as cp,
            tc.tile_pool(name="agg", bufs=4, space="PSUM") as aggp,
            tc.tile_pool(name="pha", bufs=4, space="PSUM") as phap,
        ):
            ident = constp.tile([128, 128], f16)
            make_identity(nc, ident[:])
            h_sb = constp.tile([128, NB * D], f16)
            nc.sync.dma_start(h_sb[:], h0_d.ap())
            no_sb = constp.tile([128, NB], f32)
            nc.sync.dma_start(no_sb[:], no_d.ap())
            ni_sb = constp.tile([128, NB], f32)
            nc.sync.dma_start(ni_sb[:], ni_d.ap())
            wt_sb = constp.tile([128, 3 * D], f16)
            nc.sync.dma_start(wt_sb[:], wt_d.ap())
            bb_sb = constp.tile([128, 12 * D], f16)
            nc.sync.dma_start(bb_sb[:], bb_d.ap())
            stage = constp.tile([128, ROWSPT * D], f16)
            nc.vector.memset(stage[:, NB * D:], 0.0)  # zero rows (t=98)

            mul = mybir.AluOpType.mult
            eq = mybir.AluOpType.is_equal

            def phase_a_quads(l, b_lo, b_hi):
                """Table build g_l = norm_out*(h_l @ W_l.T) for blocks
                [b_lo, b_hi), in quads of 4 blocks per psum bank."""
                for B0 in range(b_lo, b_hi, 4):
                    nb4 = min(4, b_hi - B0)
                    w4 = nb4 * D
                    psT = phap.tile([128, 4 * D], f16, tag="pha",
                                    name=f"psT{l}_{B0}")
                    for j in range(nb4):
                        B = B0 + j
                        nc.tensor.transpose(psT[:, j * D:(j + 1) * D],
                                            h_sb[:, B * D:(B + 1) * D],
                                            ident[:])
                    hT = htp.tile([128, 4 * D], f16, tag="hT",
                                  name=f"hT{l}_{B0}")
                    nc.vector.tensor_copy(hT[:, :w4], psT[:, :w4])
                    psG = phap.tile([128, 4 * D], f32, tag="pha",
                                    name=f"psG{l}_{B0}")
                    for j in range(nb4):
                        nc.tensor.matmul(psG[:, j * D:(j + 1) * D],
                                         lhsT=hT[:, j * D:(j + 1) * D],
                                         rhs=wt_sb[:, l * D:(l + 1) * D],
                                         start=True, stop=True)
                    nc.vector.tensor_tensor(
                        out=stage[:, B0 * D:B0 * D + w4].rearrange(
                            "p (b d) -> p b d", d=D),
                        in0=psG[:, :w4].rearrange("p (b d) -> p b d", d=D),
                        in1=no_sb[:, B0:B0 + nb4].to_broadcast(
                            [128, nb4, D]),
                        op=mul)

            phase_a_quads(0, 0, NB)
            for l in range(3):
                # phase A for this layer was emitted eagerly (per superblock
                # of the previous layer); ship the table.
                nc.sync.dma_start(g_local.ap(), stage[:, :])
                nc.gpsimd.collective_compute(
                    "AllGather", mybir.AluOpType.bypass,
                    replica_groups=[list(range(NCORES))],
                    ins=[g_local.ap()], outs=[table.ap()],
                )
                # ---- phase B: gather + masked-S matmul segment sum ----
                # Per superblock: issue all 4 window gathers (one SWDGE queue
                # each), then run subs BLOCK-MAJOR so each block's psum
                # accumulation group is consecutive (no interleaving within a
                # shared psum bank), then phase C for the superblock.
                sbatch_s0 = -1
                sbatch_tile = None
                for sb in range(NSB):
                    msgs_of = {}
                    for w in range(NWIN):
                        ci = sb * NWIN + w
                        ntile = call_ntile[ci]
                        if ntile == 0:
                            continue
                        ni_call = ntile * 128
                        t0 = int(tile_base[ci])
                        ixt = ixp.tile([128, ntile * 8], i16, tag="ix",
                                       name=f"ix{l}_{ci}")
                        nc.sync.dma_start(
                            ixt[:], idx_d.ap()[:, t0 * 8:(t0 + ntile) * 8])
                        msgs = msgp.tile([128, ntile * D], f16, tag="m",
                                         name=f"m{l}_{ci}")
                        nc.gpsimd.dma_gather(
                            out_ap=msgs[:].rearrange("p (t d) -> p t d", d=D),
                            in_ap=table.ap()[w * WINROWS:(w + 1) * WINROWS, :],
                            idxs_ap=ixt[:],
                            num_idxs=ni_call,
                            num_idxs_reg=ni_call,
                            elem_size=D,
                            single_packet=False,
                            queue_num=w,
                        )
                        msgs_of[w] = msgs
                    nhalf = -(-len(SB_BLOCKS[sb]) // 4)
                    halves = [aggp.tile([128, 4 * D], f32, tag="agg",
                                        name=f"agg{l}_{sb}_{h}")
                              for h in range(nhalf)]
                    slo, shi = sb_spans[sb]
                    for j in range(slo, shi):
                        ci, t, B = subs[j]
                        w = ci % NWIN
                        _, is_first, is_last = flags[j]
                        k = j % SBATCH
                        s0 = (j // SBATCH) * SBATCH
                        if s0 != sbatch_s0:
                            nb2 = min(SBATCH, NSUB - s0)
                            sbatch_tile = sp.tile(
                                [128, SBATCH * 128], f8,
                                tag="S", name=f"S{l}_{s0}")
                            nc.sync.dma_start(
                                sbatch_tile[:, :nb2 * 128],
                                sdat_d.ap()[:, s0 * 128:(s0 + nb2) * 128])
                            sbatch_s0 = s0
                        St = sbatch_tile[:, k * 128:(k + 1) * 128]
                        half, col = (B % 8) // 4, B % 4
                        nc.tensor.matmul(
                            halves[half][:, col * D:(col + 1) * D],
                            lhsT=St,
                            rhs=msgs_of[w][:, t * D:(t + 1) * D],
                            start=is_first, stop=is_last)
                    # ---- phase C for superblock sb ----
                    for h, pa in enumerate(halves):
                        B0 = sb * 8 + h * 4
                        nbh = min(4, NB - B0)
                        wd = nbh * D
                        x1 = cp.tile([128, 4 * D], f16, tag="x1",
                                     name=f"x1{l}_{sb}_{h}")
                        nc.vector.tensor_add(
                            x1[:, :wd], pa[:, :wd],
                            bb_sb[:, l * 4 * D:l * 4 * D + wd])
                        ni_b = ni_sb[:, B0:B0 + nbh].to_broadcast(
                            [128, nbh, D])
                        if l < 2:
                            x2 = cp.tile([128, 4 * D], f16, tag="x2",
                                         name=f"x2{l}_{sb}_{h}")
                            nc.vector.tensor_tensor(
                                out=x2[:, :wd].rearrange(
                                    "p (b d) -> p b d", d=D),
                                in0=x1[:, :wd].rearrange(
                                    "p (b d) -> p b d", d=D),
                                in1=ni_b, op=mul)
                            x3 = cp.tile([128, 4 * D], f16, tag="x3",
                                         name=f"x3{l}_{sb}_{h}")
                            nc.vector.tensor_add(
                                x3[:, :wd], x2[:, :wd],
                                h_sb[:, B0 * D:B0 * D + wd])
                            nc.scalar.activation(
                                h_sb[:, B0 * D:B0 * D + wd], x3[:, :wd],
                                mybir.ActivationFunctionType.Relu)
                        else:
                            nc.vector.tensor_tensor(
                                out=stage[:, B0 * D:B0 * D + wd].rearrange(
                                    "p (b d) -> p b d", d=D),
                                in0=x1[:, :wd].rearrange(
                                    "p (b d) -> p b d", d=D),
                                in1=ni_b, op=mul)
                    if l < 2:
                        # eagerly build next layer's table rows for this sb
                        phase_a_quads(l + 1, sb * 8,
                                      sb * 8 + len(SB_BLOCKS[sb]))
            nc.sync.dma_start(out_d.ap(), stage[:, :NB * D])

    nc.compile()
    return nc


_CACHE = {}


def kernel(feat, src, dst, W1, b1, W2, b2, W3, b3):
    import hashlib
    import concourse.bass_utils as bass_utils

    feat = np.asarray(feat, np.float32)
    src = np.asarray(src)
    dst = np.asarray(dst)
    key = hashlib.sha1(src.tobytes() + dst.tobytes()).hexdigest()
    if key not in _CACHE:
        meta, core_inputs = preprocess(src, dst)
        nc = build_program(meta)
        _CACHE[key] = (meta, core_inputs, nc)
    meta, core_inputs, nc = _CACHE[key]

    Wt = np.concatenate([np.asarray(w, np.float32).T for w in (W1, W2, W3)],
                        axis=1).astype(np.float16)          # [D, 3D]
    bb = np.tile(np.concatenate(
        [np.tile(np.asarray(b, np.float32), 4) for b in (b1, b2, b3)])[None],
        (128, 1)).astype(np.float16)                         # [128, 12D]

    h0_cores = _slot_rows(feat, np.float16)
    no_cores = _slot_vec(meta["norm_out"])
    ni_cores = _slot_vec(meta["norm_in"])

    in_maps = []
    for c in range(NCORES):
        idx_d, sdat = core_inputs[c]
        in_maps.append({
            "h0": h0_cores[c],
            "gidx": idx_d,
            "sdat": sdat,
            "normout": no_cores[c],
            "normin": ni_cores[c],
            "wt": Wt,
            "bb": bb,
        })

    res = bass_utils.run_bass_kernel_spmd(nc, in_maps,
                                          core_ids=list(range(NCORES)))
    out = np.zeros((N, D), np.float32)
    for c in range(NCORES):
        o = res.results[c]["out"].astype(np.float32)
        rows = o.reshape(128, NB, D).transpose(1, 0, 2).reshape(NB * 128, D)
        out[c * SHARD:(c + 1) * SHARD] = rows[:SHARD]
    return out

